# revision 26
# baseline (speedup 1.0000x reference)
"""Distributed GQA attention kernel for 8 TRN2 NeuronCores.

Problem: B=1, S=2048, D=4096, H=32 q-heads, KV=8 kv-heads, HD=128.
  q = rope(x@wq.T), k = rope(x@wk.T), v = x@wv.T
  out = softmax(causal(q@k.T/sqrt(HD))) @ v @ wo.T

Sharding: tensor-parallel over heads. Core c owns q-heads 4c..4c+3 and
kv-head c. Device-side per core:
  phase 1: QT/KT (rope'd, [hd, s] layout) + VT projections; rope runs
           off the PE (DVE muls + partition-swap DMA + DVE add); V
           tiles ([t, hd]) via DMA transpose.
  phase 2: causal attention in head PAIRS so the two M=1 rowsum
           matmuls pack into one PE slot via column tiling (psum
           partitions 0/64); softmax denominators broadcast on GpSimd;
           AllGather + out-proj scheduled so the final gather is
           covered by ~80us of deferred out-proj matmuls.
Host side: layout prep (transposes, bf16 cast, sign-folded rope
tables) + final concat/transpose of the 8 out.T slices.
"""

import math
import numpy as np
import ml_dtypes

BF = ml_dtypes.bfloat16

B, S, D = 1, 2048, 4096
H, KV, HD = 32, 8, 128
NCORES = 8
HL = H // NCORES            # 4 local q heads
QW = HL * HD                # 512 local q width
SC = 512                    # s-chunk width
NSC = S // SC               # 4 s-chunks
KD = 32                     # d-dim k-tiles (4096/128)
NT = S // 128               # 16 t-tiles
SCALE = 1.0 / math.sqrt(HD)
NEG = -30000.0

USE_GPSIMD_BC = True        # broadcast softmax denom on GpSimd (else PE)


def _build_nc():
    import concourse.bass as bass
    import concourse.mybir as mybir
    from concourse import bacc, tile

    dt = mybir.dt
    nc = bacc.Bacc()

    xt_d = nc.declare_dram_parameter("xt", [D, S], dt.bfloat16, isOutput=False)
    wqt_d = nc.declare_dram_parameter("wqt", [D, QW], dt.bfloat16, isOutput=False)
    wkt_d = nc.declare_dram_parameter("wkt", [D, HD], dt.bfloat16, isOutput=False)
    wvt_d = nc.declare_dram_parameter("wvt", [D, HD], dt.bfloat16, isOutput=False)
    wot_d = nc.declare_dram_parameter("wot", [D, QW], dt.bfloat16, isOutput=False)
    cosd_d = nc.declare_dram_parameter("cosd", [HD, S], dt.bfloat16, isOutput=False)
    sind_d = nc.declare_dram_parameter("sind", [HD, S], dt.bfloat16, isOutput=False)
    dmask_d = nc.declare_dram_parameter("dmask", [128, 128], dt.float32, isOutput=False)
    onesc_d = nc.declare_dram_parameter("onesc", [128, 1], dt.bfloat16, isOutput=False)
    onesr_d = nc.declare_dram_parameter("onesr", [1, 128], dt.bfloat16, isOutput=False)
    out_d = nc.declare_dram_parameter("out_t", [QW, S], dt.float32, isOutput=True)

    with tile.TileContext(nc) as tc:
        with (
            tc.tile_pool(name="const", bufs=1) as cpool,
            tc.tile_pool(name="qkv", bufs=1) as qkvpool,
            tc.tile_pool(name="att", bufs=1) as attpool,
            tc.tile_pool(name="dram", bufs=1, space="DRAM") as dpool,
        ):
            # ---- persistent activations ----
            qt = [qkvpool.tile([HD, S], dt.bfloat16, name=f"qt{h}", tag=f"qt{h}")
                  for h in range(HL)]
            kt = qkvpool.tile([HD, S], dt.bfloat16)
            vv = qkvpool.tile([128, NT, HD], dt.bfloat16)   # [t_part, ti, hd]
            att = [attpool.tile([HD, S], dt.bfloat16, name=f"att{h}", tag=f"att{h}")
                   for h in range(HL)]

            xt_r = xt_d[:, :].rearrange("(k p) s -> p k s", p=128)

            # small resident constants (emitted after the first x/wq pieces
            # below so those DMAs get queue-head positions)
            cosd = cpool.tile([HD, S], dt.bfloat16)
            sind = cpool.tile([HD, S], dt.bfloat16)
            dmask = cpool.tile([128, 128], dt.float32)
            onesc = cpool.tile([128, 1], dt.bfloat16)
            onesr = cpool.tile([1, 128], dt.bfloat16)

            # ================= phase 1: projections + rope =================
            with (
                tc.tile_pool(name="w1", bufs=1) as wpool,
                tc.tile_pool(name="xc", bufs=3) as xpool,
                tc.tile_pool(name="p1", bufs=6, space="PSUM") as pp1,
                tc.tile_pool(name="rtmp", bufs=2) as rtpool,
            ):
                wqt = wpool.tile([128, KD, QW], dt.bfloat16)
                wkt = wpool.tile([128, KD, HD], dt.bfloat16)
                wvt = wpool.tile([128, KD, HD], dt.bfloat16)
                vt = wpool.tile([HD, S], dt.bfloat16)
                wqt_r = wqt_d[:, :].rearrange("(k p) n -> p k n", p=128)
                wkt_r = wkt_d[:, :].rearrange("(k p) n -> p k n", p=128)
                wvt_r = wvt_d[:, :].rearrange("(k p) n -> p k n", p=128)
                # interleave the first x-chunk, wq pieces, rope tables and
                # k/v weights so everything lands just before its consumer:
                # first matmul ~2us, rope tables by ~10us (psum recycling
                # depends on the rope muls), wkt by ~35us, wvt by ~43us
                xc0 = xpool.tile([128, KD, SC], dt.bfloat16, tag="xc")

                def xq_piece(lo, hi):
                    ksl = slice(lo, hi)
                    nc.sync.dma_start(xc0[:, ksl, :], xt_r[:, ksl, 0:SC])
                    nc.sync.dma_start(wqt[:, ksl, :], wqt_r[:, ksl, :])

                xq_piece(0, 1)
                xq_piece(1, 2)
                # rope tables next (they gate psum recycling ~40us in)
                nc.sync.dma_start(cosd[:], cosd_d[:, :])
                nc.sync.dma_start(sind[:], sind_d[:, :])
                nc.sync.dma_start(dmask[:], dmask_d[:, :])
                nc.sync.dma_start(onesc[:], onesc_d[:, :])
                nc.sync.dma_start(onesr[:], onesr_d[:, :])
                xq_piece(2, 3)
                xq_piece(3, 5)
                xq_piece(5, 8)
                # k/v weights ride the ACT engine's DGE queue (idle in
                # phase 1) so the sync queue stays pure x/wq bulk
                nc.scalar.dma_start(wkt[:], wkt_r[:, :, :])
                nc.scalar.dma_start(wvt[:], wvt_r[:, :, :])
                xq_piece(8, 12)
                xq_piece(12, 18)
                xq_piece(18, 25)
                xq_piece(25, 32)
                # warm up the ACT exp table load before attention needs it
                warm = cpool.tile([1, 1], dt.float32)
                nc.scalar.activation(warm[:], dmask[0:1, 0:1],
                                     mybir.ActivationFunctionType.Exp)

                xc_next = xc0
                nxt_pieces = [(0, 6), (6, 12), (12, 18), (18, 24),
                              (24, 29), (29, 32)]
                for sc in range(NSC):
                    ssl = slice(sc * SC, (sc + 1) * SC)
                    xc = xc_next
                    if sc + 1 < NSC:
                        # prefetch next chunk's x in pieces interleaved with
                        # this chunk's matmul targets (keeps the bulk queue
                        # fed well ahead of the consumers)
                        xc_next = xpool.tile([128, KD, SC], dt.bfloat16,
                                             tag="xc")
                        nssl = slice((sc + 1) * SC, (sc + 2) * SC)

                    # 4 Q heads (rope), K (rope), V (plain) — all [hd, s]
                    for hi in range(HL + 2):
                        if sc + 1 < NSC:
                            lo, hi2 = nxt_pieces[hi]
                            nc.sync.dma_start(xc_next[:, lo:hi2, :],
                                              xt_r[:, lo:hi2, nssl])
                        ps = pp1.tile([128, SC], dt.float32)
                        for k in range(KD):
                            if hi < HL:
                                lhs = wqt[:, k, hi * HD:(hi + 1) * HD]
                            elif hi == HL:
                                lhs = wkt[:, k, :]
                            else:
                                lhs = wvt[:, k, :]
                            nc.tensor.matmul(ps[:], lhs, xc[:, k, :],
                                             start=(k == 0), stop=(k == KD - 1))
                        if hi == HL + 1:
                            nc.vector.tensor_copy(vt[:, ssl], ps[:])
                            continue
                        # rope off-PE. q/k head rows are de-interleaved
                        # host-side (real parts rows 0-63, imag rows 64-127;
                        # scores are permutation-invariant over hd), so the
                        # rotate-half is two contiguous half-tile DMAs:
                        #   out = t*cos2 + swap_halves(t*sin2)
                        # with sin2 sign-folded (+s top half, -s bottom).
                        qc = rtpool.tile([128, SC], dt.bfloat16, tag="ropeqc")
                        qs = rtpool.tile([128, SC], dt.bfloat16, tag="ropeqs")
                        qw = rtpool.tile([128, SC], dt.bfloat16, tag="ropeqw")
                        nc.vector.tensor_mul(qc[:], ps[:], cosd[:, ssl])
                        nc.vector.tensor_mul(qs[:], ps[:], sind[:, ssl])
                        # compute-dependent DMAs go on the ACT engine's DGE
                        # queue: they'd head-of-line block the bulk loads on
                        # the sync queue while waiting for the DVE muls
                        nc.scalar.dma_start(qw[0:64, :], qs[64:128, :])
                        nc.scalar.dma_start(qw[64:128, :], qs[0:64, :])
                        dst = qt[hi] if hi < HL else kt
                        nc.vector.tensor_add(dst[:, ssl], qc[:], qw[:])

                    # V tiles in [t, hd] layout via DMA transpose
                    for vtile in range(4):
                        ti = sc * 4 + vtile
                        nc.scalar.dma_start_transpose(
                            vv[:, ti, :], vt[:, ti * 128:(ti + 1) * 128])

            # ============ phase 2+3: attention, allgather, out-proj ============
            with (
                tc.tile_pool(name="wo", bufs=1) as wopool,
                tc.tile_pool(name="agc", bufs=2) as agpool,
                tc.tile_pool(name="st", bufs=3, space="PSUM") as stpool,
                tc.tile_pool(name="pv", bufs=3, space="PSUM") as pvpool,
                tc.tile_pool(name="rs", bufs=1, space="PSUM") as rspool,
                tc.tile_pool(name="p3", bufs=1, space="PSUM") as pp3,
                tc.tile_pool(name="pt", bufs=8) as ptpool,
                tc.tile_pool(name="ep", bufs=3) as eppool,
                tc.tile_pool(name="ep1", bufs=1) as ep1pool,
                tc.tile_pool(name="o3", bufs=3) as opool,
            ):
                wot = wopool.tile([128, KD, QW], dt.bfloat16)
                nc.sync.dma_start(
                    wot[:], wot_d[:, :].rearrange("(k p) n -> p k n", p=128))

                # zero-dependency dummy gather (uninitialized data, result
                # unused) to absorb first-collective setup during phase 1
                cw_in = dpool.tile([128, 16], dt.bfloat16)
                cw_out = dpool.tile([NCORES * 128, 16], dt.bfloat16,
                                    addr_space="Shared")
                nc.gpsimd.collective_compute(
                    "AllGather",
                    mybir.AluOpType.bypass,
                    replica_groups=[list(range(NCORES))],
                    ins=[cw_in.opt()],
                    outs=[cw_out.opt()],
                )

                def epilogue_front(sc, ha, hb, pvs, rs):
                    # emitted right at pair end: everything that releases
                    # PSUM (pv copies, rowsum reads) plus the ha reciprocal/
                    # broadcast. All DVE ops here have prompt dependencies,
                    # so nothing head-of-line blocks the DVE queue.
                    pvc = {}
                    for h in (ha, hb):
                        t = eppool.tile([128, SC], dt.bfloat16, tag="pvc")
                        nc.scalar.copy(t[:], pvs[h][:])
                        pvc[h] = t
                    # custom-DVE reciprocal only works at base partition 0,
                    # so move the col-packed partition-64 row down first
                    cp = ep1pool.tile([128, SC], dt.float32, tag="cp64")
                    nc.scalar.copy(cp[64:65, :], rs[64:65, :])
                    rowb = eppool.tile([1, SC], dt.float32, tag="row64")
                    nc.gpsimd.dma_start(rowb[:], cp[64:65, :])
                    bcss = {}
                    rec = eppool.tile([1, SC], dt.float32, tag="rec")
                    nc.vector.reciprocal_approx_fast(rec[:], rs[0:1, :])
                    recb = eppool.tile([1, SC], dt.bfloat16, tag="recb")
                    nc.vector.tensor_copy(recb[:], rec[:])
                    bcs = eppool.tile([128, SC], dt.bfloat16, tag="bcs")
                    nc.gpsimd.partition_broadcast(bcs[:], recb[:])
                    bcss[ha] = bcs
                    return (sc, ha, hb, pvc, rowb, bcss)

                def epilogue_back(sc, ha, hb, pvc, rowb, bcss):
                    # deferred one pair: the hb reciprocal (waits the row
                    # DMA) and the normalize muls (wait the broadcasts) run
                    # behind the next pair's first score batch
                    ssl = slice(sc * SC, (sc + 1) * SC)
                    rec = eppool.tile([1, SC], dt.float32, tag="rec")
                    nc.vector.reciprocal_approx_fast(rec[:], rowb[:])
                    recb = eppool.tile([1, SC], dt.bfloat16, tag="recb")
                    nc.vector.tensor_copy(recb[:], rec[:])
                    bcs = eppool.tile([128, SC], dt.bfloat16, tag="bcs")
                    nc.gpsimd.partition_broadcast(bcs[:], recb[:])
                    bcss[hb] = bcs
                    for h in (ha, hb):
                        nc.vector.tensor_mul(att[h][:, ssl], pvc[h][:],
                                             bcss[h][:])

                ag_outs = {}

                def allgather_trigger(sc):
                    # ONE AllGather per chunk: the collective has a ~16us
                    # fixed cost, so fewer+bigger ops keep the CC stream off
                    # the critical path. SBUF load of the result is separate
                    # (agc_load) so its buffer-reuse dep lands correctly.
                    ssl = slice(sc * SC, (sc + 1) * SC)
                    ag_in = dpool.tile([HL * HD, SC], dt.bfloat16,
                                       name=f"agi{sc}", tag=f"agi{sc}")
                    ag_out = dpool.tile([NCORES * HL * HD, SC], dt.bfloat16,
                                        name=f"ago{sc}", tag=f"ago{sc}",
                                        addr_space="Shared")
                    for h in range(HL):
                        # gpsimd software-DGE: this copy waits on the att
                        # normalize and would block agc loads on sync
                        nc.gpsimd.dma_start(ag_in[h * HD:(h + 1) * HD, :],
                                            att[h][:, ssl])
                    nc.gpsimd.collective_compute(
                        "AllGather",
                        mybir.AluOpType.bypass,
                        replica_groups=[list(range(NCORES))],
                        ins=[ag_in.opt()],
                        outs=[ag_out.opt()],
                    )
                    ag_outs[sc] = ag_out

                def agc_load(sc):
                    ag_r = ag_outs[sc][:, :].rearrange("(m p) s -> p m s",
                                                       p=128)
                    agc = agpool.tile([128, NCORES * HL, SC], dt.bfloat16,
                                      tag="agc")
                    nc.sync.dma_start(agc[:], ag_r[:, :, :])
                    return agc

                def outproj(sc, agc):
                    # block m of the gather is global head r*HL+h = m, which
                    # is exactly wot's k-tile index
                    ssl = slice(sc * SC, (sc + 1) * SC)
                    for oc in range(4):
                        ps = pp3.tile([128, SC], dt.float32, tag="ps3")
                        for m in range(NCORES * HL):
                            nc.tensor.matmul(
                                ps[:], wot[:, m, oc * 128:(oc + 1) * 128],
                                agc[:, m, :],
                                start=(m == 0), stop=(m == NCORES * HL - 1))
                        ot = opool.tile([128, SC], dt.float32, tag="ot")
                        nc.vector.tensor_copy(ot[:], ps[:])
                        nc.sync.dma_start(out_d[oc * 128:(oc + 1) * 128, ssl],
                                          ot[:])

                def scores_exp(sc, h, ti):
                    # emits scores matmul + diag mask + exp; returns (pt, v0)
                    d_off = ti * 128 - sc * SC
                    v0 = max(d_off, 0)
                    vsl = slice(v0, SC)
                    qcl = slice(sc * SC + v0, (sc + 1) * SC)
                    st = stpool.tile([128, SC], dt.float32, tag="st")
                    nc.tensor.matmul(st[:, vsl],
                                     kt[:, ti * 128:(ti + 1) * 128],
                                     qt[h][:, qcl], start=True, stop=True)
                    if d_off >= 0:
                        nc.vector.tensor_add(st[:, d_off:d_off + 128],
                                             st[:, d_off:d_off + 128],
                                             dmask[:])
                    pt = ptpool.tile([128, SC], dt.bfloat16, tag="pt")
                    nc.scalar.activation(pt[:, vsl], st[:, vsl],
                                         mybir.ActivationFunctionType.Exp,
                                         scale=SCALE)
                    return pt, v0

                # pair-interleaved attention: the two heads' M=1 rowsum
                # matmuls go to psum partitions 0/64 of one tile, so the PE
                # runs them concurrently via column tiling. Out-projs all run
                # after attention; the CC gathers pipeline underneath.
                LOOKAHEAD = 2
                pending_ep = None      # deferred epilogue_back args
                pending_ag = None      # chunk whose AG awaits epilogue_back
                agcs = {}

                for sc in range(NSC):
                    n_t = sc * 4 + 4
                    for pidx, (ha, hb) in enumerate([(0, 1), (2, 3)]):
                        cache = {}
                        emitted = 0
                        pvs = {h: pvpool.tile([128, SC], dt.float32,
                                              name=f"pv{h}", tag="pv")
                               for h in (ha, hb)}
                        rs = rspool.tile([128, SC], dt.float32,
                                         name="rs", tag="rs")
                        for ti in range(n_t):
                            while emitted <= min(ti + LOOKAHEAD, n_t - 1):
                                for h in (ha, hb):
                                    cache[(h, emitted)] = scores_exp(sc, h, emitted)
                                emitted += 1
                            if ti == 1 and pending_ep is not None:
                                epilogue_back(*pending_ep)
                                pending_ep = None
                            if ti == 3 and pending_ag is not None:
                                allgather_trigger(pending_ag)
                                if pending_ag == 2:
                                    # outproj(0) BEFORE the agc(2) load:
                                    # agc bufs=2, so the load reuses agc(0)'s
                                    # buffer and must depend on outproj(0)
                                    outproj(0, agcs[0])
                                agcs[pending_ag] = agc_load(pending_ag)
                                pending_ag = None
                            pta, v0a = cache.pop((ha, ti))
                            ptb, v0b = cache.pop((hb, ti))
                            fl = dict(start=(ti == 0), stop=(ti == n_t - 1))
                            nc.tensor.matmul(rs[0:1, v0a:], onesc[:],
                                             pta[:, v0a:], skip_group_check=True,
                                             **fl)
                            nc.tensor.matmul(rs[64:65, v0b:], onesc[:],
                                             ptb[:, v0b:], skip_group_check=True,
                                             **fl)
                            nc.tensor.matmul(pvs[ha][:, v0a:], vv[:, ti, :],
                                             pta[:, v0a:], **fl)
                            nc.tensor.matmul(pvs[hb][:, v0b:], vv[:, ti, :],
                                             ptb[:, v0b:], **fl)
                        pending_ep = epilogue_front(sc, ha, hb, pvs, rs)
                        if pidx == 1:
                            pending_ag = sc
                # final pair's epilogue + last gather trigger, then the
                # remaining out-projs; the CC stream catches up during the
                # ~100us of out-proj matmuls. agc(3)'s load sits after
                # outproj(1) so its buffer reuse (of agc(1)) is safe.
                epilogue_back(*pending_ep)
                allgather_trigger(3)
                outproj(1, agcs[1])
                agcs[3] = agc_load(3)
                outproj(2, agcs[2])
                outproj(3, agcs[3])
    if not nc.is_finalized():
        nc.finalize()
    return nc


_CACHE = {}


def _get_nc():
    if "nc" not in _CACHE:
        _CACHE["nc"] = _build_nc()
    return _CACHE["nc"]


def _prep_in_maps(x, wq, wk, wv, wo, freqs_cos, freqs_sin):
    xt = np.ascontiguousarray(x.reshape(S, D).T).astype(BF)
    # rope rows are de-interleaved: real lanes -> rows 0-63, imag -> 64-127
    # (wq/wk output rows permuted to match; scores are invariant since q and
    # k share the permutation). sin is sign-folded: +s top half, -s bottom.
    ct = np.asarray(freqs_cos, np.float32).T   # [HD//2, S]
    st = np.asarray(freqs_sin, np.float32).T
    cosd = np.concatenate([ct, ct], axis=0).astype(BF)
    sind = np.concatenate([st, -st], axis=0).astype(BF)
    t_idx = np.arange(128)[:, None]
    s_idx = np.arange(128)[None, :]
    dmask = np.where(s_idx >= t_idx, 0.0, NEG).astype(np.float32)
    onesc = np.ones((128, 1), np.float32).astype(BF)
    onesr = np.ones((1, 128), np.float32).astype(BF)

    deint = np.concatenate([np.arange(0, HD, 2), np.arange(1, HD, 2)])
    wq = np.asarray(wq, np.float32).reshape(H, HD, D)[:, deint, :].reshape(H * HD, D)
    wk = np.asarray(wk, np.float32).reshape(KV, HD, D)[:, deint, :].reshape(KV * HD, D)
    wv = np.asarray(wv, np.float32)
    wo = np.asarray(wo, np.float32)

    in_maps = []
    for c in range(NCORES):
        qsl = slice(QW * c, QW * (c + 1))
        ksl = slice(HD * c, HD * (c + 1))
        in_maps.append({
            "xt": xt,
            "wqt": np.ascontiguousarray(wq[qsl].T).astype(BF),
            "wkt": np.ascontiguousarray(wk[ksl].T).astype(BF),
            "wvt": np.ascontiguousarray(wv[ksl].T).astype(BF),
            "wot": np.ascontiguousarray(wo[qsl].T).astype(BF),
            "cosd": cosd, "sind": sind,
            "dmask": dmask, "onesc": onesc, "onesr": onesr,
        })
    return in_maps


def run(inputs, trace=False):
    from concourse.bass_utils import run_bass_kernel_spmd
    nc = _get_nc()
    in_maps = _prep_in_maps(
        inputs["x"], inputs["wq"], inputs["wk"], inputs["wv"], inputs["wo"],
        inputs["freqs_cos"], inputs["freqs_sin"])
    res = run_bass_kernel_spmd(nc, in_maps, core_ids=list(range(NCORES)),
                               trace=trace)
    shards = [np.asarray(res.results[c]["out_t"], np.float32)
              for c in range(NCORES)]
    full = np.concatenate(shards, axis=0)          # [4096, 2048]
    out = np.ascontiguousarray(full.T)[None]       # [1, 2048, 4096]
    return out.astype(np.float32), res


def kernel(**inputs):
    out, _ = run(inputs, trace=False)
    return out
